# revision 26
# baseline (speedup 1.0000x reference)
"""IsoMaxPlus distance head on 8 NeuronCores — hand-written Bass/Tile kernel.

out[n, c] = -|ds| * sqrt(max(2 - 2 * <f_n/|f_n|, p_c/|p_c|>, eps))

Sharding: 4x2 grid — feature rows sharded 4 ways, prototype rows 2 ways;
each core computes a [4096, 4096] output block, no collectives.  Versus
pure data-parallel this halves the replicated-prototype HBM traffic
(per-core HBM: 32 MB f + 32 MB p in, 32 MB fp16 out) and makes both fp8
operand tensors small enough to keep fully SBUF-resident (8 MB + 8 MB).

Per-core algorithm (N_loc=4096, D=2048, C_loc=4096):
  1. f/p rows stream in 128-row fp32 chunks on the ACT HWDGE ring.  Row
     sum-of-squares via a Square+accumulate activation (ScalarE), then
     sqrt (ScalarE) and reciprocal (VectorE).  f chunks are cast to bf16
     (VectorE), p chunks are normalized+cast in one VectorE op; both are
     transposed to the contraction layout with SBUF->SBUF xbar
     DMA-transposes (SP ring) and repacked *16 to fp8(e4m3) (VectorE)
     into per-chunk resident tiles (exact scheduler dependencies, no
     false whole-tile hazards).  f row norms fold into the epilogue.
  2. Matmul: 2 phases x 32 m-tiles traversed as a 2D wavefront (phase 1
     lags phase 0 by SHIFT=8 m-tiles) so loads, transposes and stores
     stream at a near-constant rate for the whole kernel.  Per (phase,
     m-tile): one 4-bank PSUM tile, 4x8 fp8 DoubleRow MMs
     ([128,2,128]x[128,2,512], K=256 per step).  HW note: LDWEIGHTS
     serializes with the MM in DoubleRow mode (~454 ns per K=256 MM);
     weight-reuse orderings measure identically, so the bank-outer
     order is kept.
  3. Epilogue: one ScalarE activation per (phase, m-tile) reads the
     4-bank PSUM tile and writes |out| = sqrt(psum*scale_m + 2*ds^2)
     directly as the fp16 output tile (scale_m = -2*ds^2/(256*||f_n||)
     folds the fp8 scaling, the f norm and ds).  One 0.5 MB store per
     (phase, m-tile) on the SP ring.  The host negates and upcasts
     (distances are sign-definite), so no VectorE pass touches the
     output at all.

Accuracy vs the fp32 reference: ~2.5e-3 max relative error (fp8 e4m3
rounding of the scaled operands dominates; fp16 output adds ~5e-4).

Measured (axon, 8 cores, pipelined Theil-Sen slope): ~1.0-1.2 ms/call,
at the axon dispatch floor — a 1/4-size build measures the same, so
true device time is below the floor (~0.95 ms est., LDW+MM-bound).
"""

import functools
import sys

sys.path.insert(0, "/opt/trn_rl_repo")

import numpy as np

import jax
from jax.experimental.shard_map import shard_map
from jax.sharding import Mesh, NamedSharding, PartitionSpec as P

import concourse.bass as bass
import concourse.mybir as mybir
import concourse.tile as tile
from concourse.bass2jax import bass_jit

N_CORES = 8
R_SHARDS = 4              # feature-row shards
Q_SHARDS = 2              # prototype-row shards
N_SHARD_ROWS = 16384 // R_SHARDS
C_SHARD_COLS = 8192 // Q_SHARDS
PART = 128
F32 = mybir.dt.float32
F16 = mybir.dt.float16
BF16 = mybir.dt.bfloat16
FP8 = mybir.dt.float8e4
AF = mybir.ActivationFunctionType
ALU = mybir.AluOpType

SCALE_F8 = 16.0           # keeps normalized values out of fp8 denormals
OUT_DT = F16              # device output dtype (host upcasts to fp32)


def build_iso_kernel(tc, out, f, p, ds):
    """Emit the per-core kernel. out: [N_loc, C_loc] f16 (MAGNITUDE of the
    result; the host negates), f: [N_loc, D] f32; p: [C_loc, D] f32;
    ds: [1] f32. All APs over DRAM tensors."""
    nc = tc.nc
    n_loc, d = f.shape
    c, d2 = p.shape
    assert d == d2 and n_loc % PART == 0 and d % PART == 0
    kt = d // PART            # K tiles along contraction (16)
    assert kt % 2 == 0
    kt8 = kt // 2             # fp8 DoubleRow K-steps (8)
    mt = n_loc // PART        # M tiles (32)
    fct = mt                  # f chunks (128 rows each)
    cct = c // PART           # p chunks (32)
    cb = 512                  # c-block width (one PSUM bank)
    ncb = c // cb             # 8
    PH = 4                    # c-blocks per phase (one 4-bank PSUM tile)
    nph = ncb // PH           # 2 phases
    epi_mul = -2.0 / (SCALE_F8 * SCALE_F8)

    import contextlib

    with contextlib.ExitStack() as ctx:
        consts = ctx.enter_context(tc.tile_pool(name="consts", bufs=1))
        nat = ctx.enter_context(tc.tile_pool(name="nat", bufs=4))
        cast = ctx.enter_context(tc.tile_pool(name="cast", bufs=3))
        sqs = ctx.enter_context(tc.tile_pool(name="sqs", bufs=1))
        small = ctx.enter_context(tc.tile_pool(name="small", bufs=8))
        tstage = ctx.enter_context(tc.tile_pool(name="tstage", bufs=3))
        ft8p = ctx.enter_context(tc.tile_pool(name="ft8p", bufs=1))
        pt8p = ctx.enter_context(tc.tile_pool(name="pt8p", bufs=1))
        opool = ctx.enter_context(tc.tile_pool(name="opool", bufs=2))
        pspool = ctx.enter_context(tc.tile_pool(name="pspool", bufs=2, space="PSUM"))

        # ---- constants ----
        ds_b = consts.tile([PART, 1], F32, tag="ds_b")
        ds_bcast = bass.AP(tensor=ds.tensor, offset=ds.offset, ap=[[0, PART], [1, 1]])
        nc.gpsimd.dma_start(out=ds_b, in_=ds_bcast)
        ds2b = consts.tile([PART, 1], F32, tag="ds2b")
        nc.vector.tensor_tensor(ds2b, ds_b, ds_b, ALU.mult)        # ds^2
        bias2ds = consts.tile([PART, 1], F32, tag="bias2ds")
        nc.vector.tensor_scalar_mul(bias2ds, ds2b, 2.0)            # 2*ds^2
        zero = consts.tile([PART, 1], F32, tag="zero")
        nc.vector.memset(zero, 0.0)
        # scale_ds[:, m] = ds^2 * epi_mul / ||f_row||
        scale_ds = consts.tile([PART, mt], F32, tag="scale_ds")

        # resident fp8 operands, one tile per f chunk / p c-block so the
        # scheduler sees exact dependencies (no false whole-tile hazards)
        fT8c = [ft8p.tile([PART, kt8, 2, PART], FP8, tag=f"fT8_{i}", name=f"fT8_{i}")
                for i in range(fct)]
        pT8b = [pt8p.tile([PART, kt8, 2, cb], FP8, tag=f"pT8_{b}", name=f"pT8_{b}")
                for b in range(ncb)]

        def load_chunk(src, i):
            """DMA rows [i*128, (i+1)*128) of src into a [128, d] tile."""
            t = nat.tile([PART, d], F32, tag="nat", name="nat")
            nc.scalar.dma_start(out=t, in_=src[i * PART:(i + 1) * PART, :])
            return t

        def prep_f_chunk(i):
            """Norms + bf16 cast (DVE) + xbar transpose + fp8*16 repack
            (GpSimd) for f rows [i*128, (i+1)*128)."""
            fnat = load_chunk(f, i)
            ss = small.tile([PART, 1], F32, tag="small")
            sq = sqs.tile([PART, d], F32, tag="sq")
            nc.scalar.activation(out=sq, in_=fnat, func=AF.Square, accum_out=ss)
            nrm = small.tile([PART, 1], F32, tag="small")
            nc.scalar.activation(out=nrm, in_=ss, func=AF.Sqrt, bias=zero)
            inv = small.tile([PART, 1], F32, tag="small")
            nc.vector.reciprocal(inv, nrm)
            nc.vector.tensor_scalar(
                out=scale_ds[:, i:i + 1], in0=inv, scalar1=ds2b, scalar2=epi_mul,
                op0=ALU.mult, op1=ALU.mult,
            )
            fc = cast.tile([PART, d], BF16, tag="cast")
            nc.vector.tensor_copy(out=fc, in_=fnat)
            st = tstage.tile([PART, kt, PART], BF16, tag="tstage", name="fTt")
            nc.sync.dma_start_transpose(st, fc)
            nc.gpsimd.tensor_scalar_mul(
                fT8c[i][:],
                st[:].rearrange("p (k8 ko) n -> p k8 ko n", ko=2),
                SCALE_F8,
            )

        def prep_p_chunk(j):
            """Normalize+cast (DVE) + xbar transpose + fp8*16 repack (GpSimd)
            for p rows [j*128, (j+1)*128)."""
            pnat = load_chunk(p, j)
            ssp = small.tile([PART, 1], F32, tag="small")
            sqp = sqs.tile([PART, d], F32, tag="sq")
            nc.scalar.activation(out=sqp, in_=pnat, func=AF.Square, accum_out=ssp)
            nrmp = small.tile([PART, 1], F32, tag="small")
            nc.scalar.activation(out=nrmp, in_=ssp, func=AF.Sqrt, bias=zero)
            invp = small.tile([PART, 1], F32, tag="small")
            nc.vector.reciprocal(invp, nrmp)
            pc = cast.tile([PART, d], BF16, tag="cast")
            nc.vector.tensor_scalar_mul(pc, pnat, invp)
            st = tstage.tile([PART, kt, PART], BF16, tag="tstage", name="pTt")
            nc.sync.dma_start_transpose(st, pc)
            nc.gpsimd.tensor_scalar_mul(
                pT8b[j // 4][:, :, :, (j % 4) * PART:(j % 4 + 1) * PART],
                st[:].rearrange("p (k8 ko) n -> p k8 ko n", ko=2),
                SCALE_F8,
            )

        def matmul_mtile(ph, m):
            """PH c-blocks of phase ph for m-tile m into one multi-bank PSUM
            tile, one-shot ACT epilogue (sqrt -> fp16), one output store."""
            ps = pspool.tile([PART, PH, cb], F32, tag="ps")
            for bi in range(PH):
                b = ph * PH + bi
                for k8 in range(kt8):
                    nc.tensor.matmul(
                        ps[:, bi, :],
                        lhsT=fT8c[m][:, k8, :, :],
                        rhs=pT8b[b][:, k8, :, :],
                        start=(k8 == 0),
                        stop=(k8 == kt8 - 1),
                        perf_mode=mybir.MatmulPerfMode.DoubleRow,
                    )
            oo = opool.tile([PART, PH, cb], OUT_DT, tag="o", name="oo")
            # |out| = sqrt(2*ds^2 - 2*ds^2*sim); the host applies the minus.
            nc.scalar.activation(
                out=oo, in_=ps, func=AF.Sqrt,
                bias=bias2ds, scale=scale_ds[:, m:m + 1],
            )
            nc.sync.dma_start(
                out=out[m * PART:(m + 1) * PART,
                        ph * PH * cb:(ph + 1) * PH * cb].rearrange(
                            "p (b c) -> p b c", c=cb),
                in_=oo,
            )

        # ---- emission order: 2D wavefront over (phase, m) ----
        # Phase ph lags phase ph-1 by SHIFT m-tiles, so f loads, p loads,
        # transposes and output stores stream at a near-constant rate for
        # the whole kernel instead of piling into phase 0.
        SHIFT = 8
        for i in range(4):
            prep_f_chunk(i)
        for j in range(16):
            prep_p_chunk(j)
        for i in range(4, 6):
            prep_f_chunk(i)
        fc_next, pc_next = 6, 16
        for s in range(mt + SHIFT * (nph - 1)):
            for ph in range(nph):
                m = s - SHIFT * ph
                if 0 <= m < mt:
                    matmul_mtile(ph, m)
            if fc_next < fct:
                prep_f_chunk(fc_next)
                fc_next += 1
            for _ in range(2):
                if pc_next < cct:
                    prep_p_chunk(pc_next)
                    pc_next += 1


@bass_jit
def _iso_bass(nc, f, p, ds):
    out = nc.dram_tensor(
        "out", [f.shape[0], p.shape[0]], OUT_DT, kind="ExternalOutput"
    )
    with tile.TileContext(nc) as tc:
        build_iso_kernel(tc, out[:], f[:], p[:], ds[:])
    return out


@functools.cache
def _jitted():
    devices = jax.devices()[:N_CORES]
    mesh = Mesh(np.asarray(devices).reshape(R_SHARDS, Q_SHARDS), ("r", "q"))
    fn = jax.jit(
        shard_map(
            _iso_bass,
            mesh=mesh,
            in_specs=(P("r"), P("q"), P()),
            out_specs=P("r", "q"),
            check_rep=False,
        )
    )
    return fn, mesh


def _device_args(features, prototypes, distance_scale):
    fn, mesh = _jitted()
    f = jax.device_put(features, NamedSharding(mesh, P("r")))
    p = jax.device_put(prototypes, NamedSharding(mesh, P("q")))
    ds = jax.device_put(distance_scale, NamedSharding(mesh, P()))
    return fn, (f, p, ds)


def kernel(features, prototypes, distance_scale):
    features = np.ascontiguousarray(features, dtype=np.float32)
    prototypes = np.ascontiguousarray(prototypes, dtype=np.float32)
    distance_scale = np.ascontiguousarray(distance_scale, dtype=np.float32)
    fn, args = _device_args(features, prototypes, distance_scale)
    out = fn(*args)
    # device returns |result| in fp16; restore sign + fp32 here
    return -np.asarray(jax.device_get(out)).astype(np.float32)
